# revision 31
# baseline (speedup 1.0000x reference)
"""AgentAttention Trainium2 kernel (8-core data-parallel over batch).

Per core: 4 samples, channel-on-partition layouts, bf16 matmuls with fp32
PSUM accumulation. Stage-1 attention is computed transposed ([n,196] tiles)
so no transposes are needed; stage-2 runs in [98,n] pair layout with the
softmax denominator folded into the output matmul as extra ones-columns
and the 1/Z broadcast done by a tiny K=2 bf16 matmul. The depthwise 3x3
conv runs on the tensor engine as 9 diagonal matmuls over a 60-wide
zero-padded copy of v, read as flat contiguous 480-col windows (row-wrap
garbage lands in pad columns). Elementwise work is spread across Vector /
Scalar / GpSimd(Pool) so no single engine saturates; softmax reciprocals
use the fast approximate DVE op.
"""

import numpy as np
from contextlib import ExitStack

DIM = 384
HEADS = 12
AGENT = 49
POOL = 7
HW = 56
N = HW * HW            # 3136
HD = DIM // HEADS      # 32
SCALE = HD ** -0.5
B = 32
NCORES = 8
BPC = B // NCORES      # 4
NCH = DIM // 128       # 3
NCB = 25
CB_SIZES = [128] * 24 + [64]
CB_OFFS = [128 * i for i in range(25)]
NGRP = 3
NPAIR = 6
GW = 256               # 4 heads x 64-aligned 49-slots
PW = 98                # 2*49
VSLOT = 390            # 6 x (64 v-cols + 1 ones-col)
PADW = 58 * 60         # 3480: 58 rows x 60 cols padded image
# 1 leading + 8 trailing spare cols so every flat 480-col dwc window
# (start = 1 + c*PADW + (r0+dy+1)*60 + dx, dx in {-1,0,1}) stays in-tile
PADX = 1 + NCH * PADW + 8

_BUILT = None


def _bilinear_weights(out_size, in_size):
    w = np.zeros((out_size, in_size), np.float64)
    scale = in_size / out_size
    for i in range(out_size):
        src = (i + 0.5) * scale - 0.5
        s0 = int(np.floor(src))
        frac = src - s0
        s0c = min(max(s0, 0), in_size - 1)
        s1c = min(max(s0 + 1, 0), in_size - 1)
        w[i, s0c] += 1.0 - frac
        w[i, s1c] += frac
    return w


def _upsample_bias(b77):
    W = _bilinear_weights(HW, POOL)
    t = np.einsum("yk,hakl,xl->hayx", W, b77.astype(np.float64), W)
    return t.astype(np.float32)


def _build_program(reps=1):
    import concourse.bass as bass
    import concourse.tile as tile
    from concourse import mybir

    dt = mybir.dt
    F32 = dt.float32
    BF16 = dt.bfloat16
    AF = mybir.ActivationFunctionType
    ALU = mybir.AluOpType
    AX = mybir.AxisListType

    nc = bass.Bass("TRN2", target_bir_lowering=False, debug=False,
                   num_devices=NCORES)

    x_d = nc.dram_tensor("x4", [BPC, DIM, N], F32, kind="ExternalInput").ap()
    xb_d = nc.dram_tensor("xb4", [BPC, DIM, N], BF16,
                          kind="ExternalInput").ap()
    qkvw_d = nc.dram_tensor("qkvw", [NCH, 128, 3 * DIM], BF16,
                            kind="ExternalInput").ap()
    projw_d = nc.dram_tensor("projw", [NCH, 128, DIM], BF16,
                             kind="ExternalInput").ap()
    qkvb_d = nc.dram_tensor("qkvb", [9, 128], F32, kind="ExternalInput").ap()
    projb_d = nc.dram_tensor("projb", [NCH, 128], F32,
                             kind="ExternalInput").ap()
    dwcb_d = nc.dram_tensor("dwcb", [NCH, 128], F32,
                            kind="ExternalInput").ap()
    pb1_d = nc.dram_tensor("pb1texp", [NGRP, NCB, 128, GW], BF16,
                           kind="ExternalInput").ap()
    ab1_d = nc.dram_tensor("ab1exp", [NPAIR, 128, N], BF16,
                           kind="ExternalInput").ap()
    dwcdiag_d = nc.dram_tensor("dwcdiag", [NCH, 9, 128, 128], BF16,
                               kind="ExternalInput").ap()
    dwcw_d = nc.dram_tensor("dwcw", [NCH * 9, 128], F32,
                            kind="ExternalInput").ap()
    ident_d = nc.dram_tensor("ident", [128, 128], BF16,
                             kind="ExternalInput").ap()
    avpi_d = nc.dram_tensor("avpinit", [128, NPAIR * 66], BF16,
                            kind="ExternalInput").ap()
    pairE_d = nc.dram_tensor("pairE", [2, 64], BF16,
                             kind="ExternalInput").ap()
    y_d = nc.dram_tensor("y4", [BPC, DIM, N], F32, kind="ExternalOutput").ap()

    NT = [(0, 1024), (1024, 1024), (2048, 1024), (3072, 64)]

    with tile.TileContext(nc) as tc, ExitStack() as ctx:
        const = ctx.enter_context(tc.tile_pool(name="const", bufs=1))
        qkvw = const.tile([128, NCH * 3 * DIM], BF16, tag="qkvw")
        for kc in range(NCH):
            nc.sync.dma_start(qkvw[:, kc * 3 * DIM:(kc + 1) * 3 * DIM],
                              qkvw_d[kc])
        projw = const.tile([128, NCH * DIM], BF16, tag="projw")
        for kc in range(NCH):
            nc.sync.dma_start(projw[:, kc * DIM:(kc + 1) * DIM], projw_d[kc])
        qkvb = const.tile([128, 9], F32, tag="qkvb")
        nc.sync.dma_start(qkvb[:], qkvb_d.transpose([1, 0]))
        projb = const.tile([128, NCH], F32, tag="projb")
        nc.sync.dma_start(projb[:], projb_d.transpose([1, 0]))
        dwcb = const.tile([128, NCH], F32, tag="dwcb")
        nc.sync.dma_start(dwcb[:], dwcb_d.transpose([1, 0]))
        dwcdiag = const.tile([128, NCH * 9 * 128], BF16, tag="dwcdiag")
        for c in range(NCH):
            for t in range(9):
                nc.sync.dma_start(
                    dwcdiag[:, (c * 9 + t) * 128:(c * 9 + t + 1) * 128],
                    dwcdiag_d[c, t])
        dwcw = const.tile([128, NCH * 9], F32, tag="dwcw")
        nc.sync.dma_start(dwcw[:], dwcw_d.transpose([1, 0]))
        ident = const.tile([128, 128], BF16, tag="ident")
        nc.sync.dma_start(ident[:], ident_d)
        pairE = const.tile([2, 64], BF16, tag="pairE")
        nc.sync.dma_start(pairE[:], pairE_d)

        qpool = ctx.enter_context(tc.tile_pool(name="qpool", bufs=1))
        kpool = ctx.enter_context(tc.tile_pool(name="kpool", bufs=1))
        vpool = ctx.enter_context(tc.tile_pool(name="vpool", bufs=1))
        vbig = ctx.enter_context(tc.tile_pool(name="vbig", bufs=1))
        big1 = ctx.enter_context(tc.tile_pool(name="big1", bufs=1))
        u2pool = ctx.enter_context(tc.tile_pool(name="u2p", bufs=2))
        pb1pool = ctx.enter_context(tc.tile_pool(name="pb1p", bufs=1))
        ab1pool = ctx.enter_context(tc.tile_pool(name="ab1p", bufs=2))
        small = ctx.enter_context(tc.tile_pool(name="small", bufs=1))
        tiny = ctx.enter_context(tc.tile_pool(name="tiny", bufs=2))
        tiny1 = ctx.enter_context(tc.tile_pool(name="tiny1", bufs=1))
        xfp = ctx.enter_context(tc.tile_pool(name="xfp", bufs=2))
        xbp = ctx.enter_context(tc.tile_pool(name="xbp", bufs=4))
        accp = ctx.enter_context(tc.tile_pool(name="accp", bufs=1))
        ywp = ctx.enter_context(tc.tile_pool(name="ywp", bufs=2))
        psA = ctx.enter_context(
            tc.tile_pool(name="psA", bufs=3, space="PSUM"))
        psS = ctx.enter_context(
            tc.tile_pool(name="psS", bufs=2, space="PSUM"))

        for _rep in range(reps):
          # proj+gate of sample s is emitted AFTER phase A of sample s+1
          # (rotated software pipeline) so the PE rolls straight from the
          # dwc of one sample into the qkv of the next.
          pending_tail = None
          for s in range(BPC):
            # ---------- phase A: qkv (+ incremental agent pooling) ----------
            q_sb = qpool.tile([128, NCH * N], BF16, tag="q")
            k_sb = kpool.tile([128, NCH * N], BF16, tag="k")
            v_sb = vpool.tile([128, NCH * N], BF16, tag="v")
            agent = small.tile([128, NCH * AGENT], F32, tag="agent")
            p1 = small.tile([128, NCH * 392], F32, tag="pool1")
            dst = {0: q_sb, 1: k_sb, 2: v_sb}
            for (no, nsz) in NT:
                xw = []
                for kc in range(NCH):
                    xb = xbp.tile([128, 1024], BF16, tag="xb")
                    nc.gpsimd.dma_start(
                        xb[:, :nsz],
                        xb_d[s, kc * 128:(kc + 1) * 128, no:no + nsz])
                    xw.append(xb)
                for mc in range(9):
                    ps = psA.tile([128, 1024], F32, tag="mm")
                    for kc in range(NCH):
                        for h in range(0, nsz, 512):
                            hsz = min(512, nsz - h)
                            nc.tensor.matmul(
                                ps[:, h:h + hsz],
                                qkvw[:, kc * 3 * DIM + mc * 128:
                                     kc * 3 * DIM + mc * 128 + 128],
                                xw[kc][:, h:h + hsz],
                                start=(kc == 0), stop=(kc == NCH - 1))
                    out_sl = dst[mc // 3][:, (mc % 3) * N + no:
                                          (mc % 3) * N + no + nsz]
                    if mc // 3 == 2:
                        nc.scalar.activation(out_sl, ps[:, :nsz], AF.Identity,
                                             bias=qkvb[:, mc:mc + 1])
                    else:
                        nc.vector.tensor_scalar_add(out_sl, ps[:, :nsz],
                                                    qkvb[:, mc:mc + 1])
                    if mc < NCH:
                        # pooling step 1 for this q chunk, in the shadow of
                        # the remaining qkv matmuls
                        nc.vector.tensor_reduce(
                            p1[:, mc * 392 + no // 8:
                               mc * 392 + (no + nsz) // 8],
                            out_sl.rearrange("p (a b) -> p a b", b=8),
                            axis=AX.X, op=ALU.add)

            if pending_tail is not None:
                pending_tail()
                pending_tail = None

            # ---------- phase B: pooling step 2 + block-diag builds ----------
            for c in range(NCH):
                src = p1[:, c * 392:(c + 1) * 392].rearrange(
                    "p (py y8 px) -> p py y8 px", y8=8, px=7)
                nc.vector.tensor_reduce(
                    agent[:, c * AGENT:(c + 1) * AGENT],
                    src.transpose([0, 1, 3, 2]),
                    axis=AX.X, op=ALU.add)
            ahBD = small.tile([128, NGRP * GW], BF16, tag="ahBD")
            nc.gpsimd.memset(ahBD[:], 0.0)
            for g in range(NGRP):
                for j in range(4):
                    nc.vector.tensor_scalar_mul(
                        ahBD[32 * j:32 * (j + 1),
                             g * GW + 64 * j:g * GW + 64 * j + AGENT],
                        agent[32 * j:32 * (j + 1),
                              g * AGENT:(g + 1) * AGENT],
                        1.0 / 64.0)
            ahP = small.tile([128, NPAIR * 128], BF16, tag="ahP")
            nc.gpsimd.memset(ahP[:], 0.0)
            for p in range(NPAIR):
                for j in range(2):
                    r0 = (64 * p + 32 * j) % 128
                    cc = p // 2
                    nc.vector.tensor_scalar_mul(
                        ahP[r0:r0 + 32,
                            p * 128 + 64 * j:p * 128 + 64 * j + AGENT],
                        agent[r0:r0 + 32, cc * AGENT:(cc + 1) * AGENT],
                        1.0 / 64.0)

            # ---------- v transpose (v_nc) ----------
            v_nc = vbig.tile([128, NCB * VSLOT], BF16, tag="vb")
            nc.vector.memset(
                v_nc[:].rearrange("p (cb pr o) -> p cb pr o", pr=NPAIR,
                                  o=65)[:, :, :, 64:65],
                1.0)
            for c in range(NCH):
                for cb in range(NCB):
                    csz = CB_SIZES[cb]
                    pt = psS.tile([128, 512], BF16, tag="sm")
                    nc.tensor.transpose(
                        pt[:csz, :128],
                        v_sb[:, c * N + CB_OFFS[cb]:c * N + CB_OFFS[cb] + csz],
                        ident[:])
                    ev_dst = v_nc[:csz, cb * VSLOT + c * 130:
                                  cb * VSLOT + (c + 1) * 130].rearrange(
                        "p (pr o) -> p pr o", o=65)[:, :, 0:64]
                    ev_src = pt[:csz, :128].rearrange(
                        "p (pr o) -> p pr o", o=64)
                    if cb % 2 == 0:
                        nc.vector.tensor_copy(ev_dst, ev_src)
                    else:
                        nc.scalar.copy(ev_dst, ev_src)

            # ---------- phase C: stage-1 attention ----------
            u1 = big1.tile([128, NCB * GW], BF16, tag="b1")
            avP = small.tile([128, NPAIR * 66], BF16, tag="avP")
            nc.gpsimd.dma_start(avP[:], avpi_d)
            for g in range(NGRP):
                pb1t = pb1pool.tile([128, NCB * GW], BF16, tag="pb1t")
                nc.sync.dma_start(
                    pb1t[:].rearrange("p (a b) -> p a b", b=GW),
                    pb1_d[g].transpose([1, 0, 2]))
                done = 0
                while done < NCB:
                    cnt = min(4, NCB - done)
                    ps = psA.tile([128, 1024], F32, tag="mm")
                    offs = [0, 256, 512, 768]
                    for i in range(cnt):
                        cb = done + i
                        csz = CB_SIZES[cb]
                        nc.tensor.matmul(
                            ps[:csz, offs[i]:offs[i] + GW],
                            k_sb[:, g * N + CB_OFFS[cb]:
                                 g * N + CB_OFFS[cb] + csz],
                            ahBD[:, g * GW:(g + 1) * GW],
                            start=True, stop=True)
                    if cnt == 4 and CB_SIZES[done + 3] == 128:
                        nc.scalar.activation(
                            u1[:, done * GW:(done + 4) * GW],
                            ps[:, :1024], AF.Exp, scale=SCALE)
                    else:
                        for i in range(cnt):
                            cb = done + i
                            csz = CB_SIZES[cb]
                            nc.scalar.activation(
                                u1[:csz, cb * GW:(cb + 1) * GW],
                                ps[:csz, offs[i]:offs[i] + GW],
                                AF.Exp, scale=SCALE)
                    done += cnt
                nc.vector.tensor_mul(u1[:, :5120], u1[:, :5120],
                                     pb1t[:, :5120])
                nc.gpsimd.tensor_mul(u1[:, 5120:], u1[:, 5120:],
                                     pb1t[:, 5120:])
                for pj in range(2):
                    p = 2 * g + pj
                    psav = psS.tile([128, 512], F32, tag="sm")
                    for cb in range(NCB):
                        csz = CB_SIZES[cb]
                        nc.tensor.matmul(
                            psav[:, 0:65],
                            u1[:csz, cb * GW + 128 * pj:
                               cb * GW + 128 * (pj + 1)],
                            v_nc[:csz, cb * VSLOT + 65 * p:
                                 cb * VSLOT + 65 * p + 65],
                            start=(cb == 0), stop=(cb == NCB - 1))
                    rz1 = tiny.tile([128, 1], F32, tag="rz1")
                    nc.vector.reciprocal(rz1[:], psav[:, 64:65])
                    av = avP[:, p * 66:(p + 1) * 66]
                    nc.vector.tensor_scalar_mul(
                        av[0:49, 0:32], psav[0:49, 0:32], rz1[0:49, :])
                    nc.vector.tensor_scalar_mul(
                        av[64:113, 32:64], psav[64:113, 32:64],
                        rz1[64:113, :])

            # ---------- v pad (for dwc): DMA interior, memset border ----------
            v_pad = vbig.tile([128, PADX], BF16, tag="vb")
            vp4 = v_pad[:, 1:1 + NCH * PADW].rearrange(
                "p (c y x) -> p c y x", y=58, x=60)
            for c in range(NCH):
                nc.gpsimd.memset(vp4[:, c, 0:1, :], 0.0)
                nc.gpsimd.memset(vp4[:, c, 57:58, :], 0.0)
                nc.gpsimd.memset(vp4[:, c, 1:57, 0:1], 0.0)
                nc.gpsimd.memset(vp4[:, c, 1:57, 57:60], 0.0)
                nc.sync.dma_start(
                    vp4[:, c, 1:57, 1:57],
                    v_sb[:, c * N:(c + 1) * N].rearrange(
                        "p (a b) -> p a b", b=56))
            nc.gpsimd.memset(v_pad[:, 0:1], 0.0)
            nc.gpsimd.memset(v_pad[:, 1 + NCH * PADW:], 0.0)

            # ---------- phase D: stage-2 + output attention ----------
            # Software-pipelined: pair p+1's scores/exp/mul are emitted
            # before pair p's output matmuls so the PE never waits on the
            # exp+mul chain of the pair it is about to consume.
            o_sb = big1.tile([128, NCH * N], BF16, tag="b1")
            u2s = {}

            def s2_scores(p):
                ab1 = ab1pool.tile([128, N], BF16, tag="ab1")
                nc.sync.dma_start(ab1[:], ab1_d[p])
                u2 = u2pool.tile([128, N], BF16, tag="u2")
                u2s[p] = u2
                g, half = p // 2, p % 2
                for w0 in range(0, N, 1024):
                    wsz = min(1024, N - w0)
                    ps = psA.tile([128, 1024], F32, tag="mm")
                    for h in range(0, wsz, 512):
                        hsz = min(512, wsz - h)
                        nc.tensor.matmul(
                            ps[:, h:h + hsz],
                            ahP[64 * half:64 * half + 64,
                                p * 128:(p + 1) * 128],
                            q_sb[64 * half:64 * half + 64,
                                 g * N + w0 + h:g * N + w0 + h + hsz],
                            start=True, stop=True)
                    nc.scalar.activation(u2[:, w0:w0 + wsz], ps[:, :wsz],
                                         AF.Exp, scale=SCALE)
                nc.vector.tensor_mul(u2[:, :2496], u2[:, :2496],
                                     ab1[:, :2496])
                nc.gpsimd.tensor_mul(u2[:, 2496:], u2[:, 2496:],
                                     ab1[:, 2496:])

            def s2_out(p):
                u2 = u2s.pop(p)
                g, half = p // 2, p % 2
                for w0 in range(0, N, 1024):
                    wsz = min(1024, N - w0)
                    pso = psA.tile([128, 1024], F32, tag="mm")
                    for h in range(0, wsz, 512):
                        hsz = min(512, wsz - h)
                        nc.tensor.matmul(pso[:66, h:h + hsz],
                                         avP[:, p * 66:(p + 1) * 66],
                                         u2[:, w0 + h:w0 + h + hsz],
                                         start=True, stop=True)
                    lnz = tiny.tile([2, 1024], F32, tag="lnz")
                    nc.scalar.activation(lnz[:, :wsz], pso[64:66, :wsz],
                                         AF.Ln)
                    rz2b = tiny.tile([2, 1024], BF16, tag="rz2b")
                    nc.scalar.activation(rz2b[:, :wsz], lnz[:, :wsz],
                                         AF.Exp, scale=-1.0)
                    psb = psA.tile([128, 1024], F32, tag="mm")
                    for h in range(0, wsz, 512):
                        hsz = min(512, wsz - h)
                        nc.tensor.matmul(psb[:64, h:h + hsz], pairE[:],
                                         rz2b[:, h:h + hsz],
                                         start=True, stop=True)
                    bsb = tiny.tile([64, 1024], BF16, tag="bsb")
                    if (w0 // 1024) % 2 == 0:
                        nc.vector.tensor_copy(bsb[:, :wsz], psb[:64, :wsz])
                    else:
                        nc.scalar.copy(bsb[:, :wsz], psb[:64, :wsz])
                    nc.vector.tensor_mul(
                        o_sb[64 * half:64 * half + 64,
                             g * N + w0:g * N + w0 + wsz],
                        pso[0:64, :wsz], bsb[:, :wsz])

            s2_scores(0)
            for p in range(NPAIR):
                if p + 1 < NPAIR:
                    s2_scores(p + 1)
                s2_out(p)

            # ---------- phase E: dwc (in-place into o_sb) ----------
            # dwc: two interleaved 9-matmul strip-chains per psA tile
            # (separate PSUM banks), up to 6 chains in flight. The evac
            # adds dwc + bias into o_sb in place, so no extra buffer.
            for c in range(NCH):
                for sp in range(4):
                    strips = [16 * sp] + ([16 * sp + 8] if sp < 3 else [])
                    psd = psA.tile([128, 1024], F32, tag="mm")
                    offs = [0, 512]
                    for t in range(9):
                        dy, dx = t // 3 - 1, t % 3 - 1
                        for i, r0 in enumerate(strips):
                            fo = 1 + c * PADW + (r0 + dy + 1) * 60 + dx
                            nc.tensor.matmul(
                                psd[:, offs[i]:offs[i] + 480],
                                dwcdiag[:, (c * 9 + t) * 128:
                                        (c * 9 + t + 1) * 128],
                                v_pad[:, fo:fo + 480],
                                start=(t == 0), stop=(t == 8))
                    for i, r0 in enumerate(strips):
                        osl = o_sb[:, c * N + r0 * 56:
                                   c * N + r0 * 56 + 448].rearrange(
                            "p (a b) -> p a b", b=56)
                        nc.vector.scalar_tensor_tensor(
                            osl,
                            psd[:, offs[i]:offs[i] + 480].rearrange(
                                "p (a b) -> p a b", b=60)[:, :, 1:57],
                            dwcb[:, c:c + 1],
                            osl,
                            op0=ALU.add, op1=ALU.add)

            def make_tail(s, o_sb):
                def tail():
                    for mc in range(NCH):
                        for (no, nsz) in NT:
                            ps = psA.tile([128, 1024], F32, tag="mm")
                            for kc in range(NCH):
                                for h in range(0, nsz, 512):
                                    hsz = min(512, nsz - h)
                                    nc.tensor.matmul(
                                        ps[:, h:h + hsz],
                                        projw[:, kc * DIM + mc * 128:
                                              kc * DIM + mc * 128 + 128],
                                        o_sb[:, kc * N + no + h:
                                             kc * N + no + h + hsz],
                                        start=(kc == 0),
                                        stop=(kc == NCH - 1))
                            sg = ywp.tile([128, 1024], BF16, tag="sg")
                            nc.scalar.activation(sg[:, :nsz], ps[:, :nsz],
                                                 AF.Sigmoid,
                                                 bias=projb[:, mc:mc + 1])
                            xf = xfp.tile([128, 1024], F32, tag="xf")
                            nc.gpsimd.dma_start(
                                xf[:, :nsz],
                                x_d[s, mc * 128:(mc + 1) * 128, no:no + nsz])
                            yw = ywp.tile([128, 1024], F32, tag="ywt")
                            nc.gpsimd.tensor_mul(yw[:, :nsz], sg[:, :nsz],
                                                 xf[:, :nsz])
                            nc.sync.dma_start(
                                y_d[s, mc * 128:(mc + 1) * 128, no:no + nsz],
                                yw[:, :nsz])
                return tail

            pending_tail = make_tail(s, o_sb)
          pending_tail()
    _cap_waits(nc, mybir)
    return nc


def _cap_waits(nc, mybir):
    """Walrus codegen allows 1 sem-wait on DMA descriptors and 2 on
    engine instructions. Tile can emit more; spill the excess onto
    EventSemaphore instructions inserted just before the offender on the
    same engine (sequencer executes them in order, so semantics hold)."""
    f = nc.m.functions[0]
    n_spill = 0
    for blk in f.blocks:
        insts = blk.instructions
        out = []
        for inst in insts:
            si = inst.sync_info
            tname = type(inst).__name__
            limit = 2 if tname == "InstEventSemaphore" else 1
            if si is not None and si.on_wait and len(si.on_wait) > limit:
                waits = list(si.on_wait)
                keep = waits[:limit]
                extra = waits[limit:]
                while extra:
                    chunk, extra = extra[:2], extra[2:]
                    ev = mybir.InstEventSemaphore(
                        name=nc.get_next_instruction_name(),
                        engine=inst.engine,
                        sync_info=mybir.SyncInfo(on_wait=chunk,
                                                 on_update=[]),
                    )
                    out.append(ev)
                    n_spill += 1
                si.on_wait = keep
            out.append(inst)
        insts[:] = out
    return n_spill


def _host_inputs(inputs):
    import ml_dtypes
    bf16 = ml_dtypes.bfloat16
    x = np.ascontiguousarray(inputs["x"].reshape(B, DIM, N), np.float32)
    qkvw = np.ascontiguousarray(
        np.asarray(inputs["qkv_w"], np.float32).reshape(
            NCH, 128, 3 * DIM)).astype(bf16)
    projw = np.ascontiguousarray(
        np.asarray(inputs["proj_w"], np.float32).reshape(
            NCH, 128, DIM)).astype(bf16)
    qkvb = np.ascontiguousarray(
        np.asarray(inputs["qkv_b"], np.float32).reshape(9, 128))
    projb = np.ascontiguousarray(
        np.asarray(inputs["proj_b"], np.float32).reshape(NCH, 128))
    dwcb = np.ascontiguousarray(
        np.asarray(inputs["dwc_b"], np.float32).reshape(NCH, 128))

    pb1 = _upsample_bias(np.asarray(inputs["an_bias"], np.float32))
    pb1 = pb1.reshape(HEADS, AGENT, N)
    pb1texp = np.zeros((NGRP, NCB, 128, GW), np.float32)
    for g in range(NGRP):
        t = np.zeros((N, GW), np.float32)
        for j in range(4):
            t[:, 64 * j:64 * j + AGENT] = np.exp(
                pb1[4 * g + j]).transpose(1, 0)
        for cb in range(NCB):
            pb1texp[g, cb, :CB_SIZES[cb]] = t[CB_OFFS[cb]:
                                              CB_OFFS[cb] + CB_SIZES[cb]]
    pb1texp = pb1texp.astype(bf16)

    ab1 = _upsample_bias(np.asarray(inputs["na_bias"], np.float32))
    ab1 = np.exp(ab1.reshape(HEADS, AGENT, N))
    ab1exp = np.zeros((NPAIR, 128, N), np.float32)
    for p in range(NPAIR):
        ab1exp[p, 0:AGENT] = ab1[2 * p]
        ab1exp[p, 64:64 + AGENT] = ab1[2 * p + 1]
    ab1exp = ab1exp.astype(bf16)

    dwc_w = np.asarray(inputs["dwc_w"], np.float32).reshape(DIM, 3, 3)
    dwcdiag = np.zeros((NCH, 9, 128, 128), np.float32)
    dwcw = np.zeros((NCH * 9, 128), np.float32)
    for c in range(NCH):
        for t in range(9):
            np.fill_diagonal(dwcdiag[c, t],
                             dwc_w[c * 128:(c + 1) * 128, t // 3, t % 3])
            dwcw[c * 9 + t] = dwc_w[c * 128:(c + 1) * 128, t // 3, t % 3]
    dwcdiag = dwcdiag.astype(bf16)

    ident = np.eye(128, dtype=np.float32).astype(bf16)
    avpi = np.zeros((128, NPAIR * 66), np.float32)
    for p in range(NPAIR):
        avpi[0:AGENT, p * 66 + 64] = 1.0
        avpi[64:64 + AGENT, p * 66 + 65] = 1.0
    avpi = avpi.astype(bf16)
    pairE = np.zeros((2, 64), np.float32)
    pairE[0, 0:32] = 1.0
    pairE[1, 32:64] = 1.0
    pairE = pairE.astype(bf16)

    shared = dict(qkvw=qkvw, projw=projw, qkvb=qkvb, projb=projb, dwcb=dwcb,
                  pb1texp=pb1texp, ab1exp=ab1exp, dwcdiag=dwcdiag,
                  dwcw=dwcw, ident=ident, avpinit=avpi, pairE=pairE)
    xb = x.astype(bf16)
    in_maps = []
    for core in range(NCORES):
        m = dict(shared)
        m["x4"] = np.ascontiguousarray(x[core * BPC:(core + 1) * BPC])
        m["xb4"] = np.ascontiguousarray(xb[core * BPC:(core + 1) * BPC])
        in_maps.append(m)
    return in_maps


def kernel(**inputs):
    global _BUILT
    from concourse.bass_utils import run_bass_kernel_spmd
    if _BUILT is None:
        _BUILT = _build_program()
    in_maps = _host_inputs(inputs)
    res = run_bass_kernel_spmd(_BUILT, in_maps, list(range(NCORES)))
    outs = [r["y4"].reshape(BPC, DIM, HW, HW) for r in res.results]
    return np.ascontiguousarray(np.concatenate(outs, axis=0))
